# revision 5
# baseline (speedup 1.0000x reference)
"""Doc2vec-style embedding lookup + negative-sampling scores on 8 trn2 cores.

reference:
    x[b, :] = D[doc_ids[b]] + sum_c W[context_ids[b, c]]      # (B, 256)
    scores[b, k] = dot(x[b], O[:, target_noise_ids[b, k]])    # (B, 6)

Strategy: data-parallel over batch (512 items/core), tables replicated.
Host concatenates [D; W; O.T] into one row table so every lookup is a row
gather from a single DRAM tensor; each batch item needs 15 rows
(1 doc + 8 ctx + 6 noise).  Per core: 4 batch-tiles of 128 items; each tile
is ONE indirect DMA gathering 128x15 rows into SBUF, then a DVE strided
tensor_reduce sums the 9 embedding rows into x, and 6 fused
tensor_tensor_reduce ops produce the dot-product scores.
"""

import sys

sys.path.insert(0, "/opt/trn_rl_repo")

import numpy as np

from concourse import bacc, bass, mybir, tile
from concourse.bass_utils import run_bass_kernel_spmd

VEC = 256
N_DOCS = 100000
N_WORDS = 50000
B = 4096
N_CTX = 8
N_NOISE = 6
N_CORES = 8
BPC = B // N_CORES  # 512 batch items per core
P = 128
TILES = BPC // P  # 4 batch tiles per core
RPI = 1 + N_CTX + N_NOISE  # 15 gathered rows per batch item
T_ROWS = N_DOCS + 2 * N_WORDS  # 200000

_nc_cache = None


def build_nc():
    nc = bacc.Bacc(None, target_bir_lowering=False, debug=True)
    tbl = nc.declare_dram_parameter(
        "tbl", [T_ROWS, VEC], mybir.dt.float32, isOutput=False
    )
    idx = nc.declare_dram_parameter(
        "idx", [P, TILES * RPI], mybir.dt.int32, isOutput=False
    )
    out = nc.declare_dram_parameter(
        "out", [P, TILES * N_NOISE], mybir.dt.float32, isOutput=True
    )

    with tile.TileContext(nc) as tc:
        with (
            tc.tile_pool(name="gpool", bufs=2) as gpool,
            tc.tile_pool(name="cpool", bufs=1) as cpool,
        ):
            idx_t = cpool.tile([P, TILES * RPI], mybir.dt.int32)
            nc.sync.dma_start(out=idx_t[:], in_=idx[:])
            score_t = cpool.tile([P, TILES * N_NOISE], mybir.dt.float32)
            for j in range(TILES):
                g = gpool.tile([P, RPI * VEC], mybir.dt.float32, tag="g")
                # HW indirect DMA uses ONE index per partition (the rest of
                # the offset AP's free dim is ignored and the descriptor just
                # reads contiguous bytes), so emit one gather per row-slot.
                for r in range(RPI):
                    col = j * RPI + r
                    nc.gpsimd.indirect_dma_start(
                        out=g[:, r * VEC : (r + 1) * VEC],
                        out_offset=None,
                        in_=tbl[:],
                        in_offset=bass.IndirectOffsetOnAxis(
                            ap=idx_t[:, col : col + 1], axis=0
                        ),
                    )
                x = gpool.tile([P, VEC], mybir.dt.float32, tag="x")
                # x[p, d] = sum_r g[p, r*VEC + d] over the 9 embedding rows
                nc.vector.tensor_reduce(
                    out=x[:],
                    in_=g[:, : (1 + N_CTX) * VEC].rearrange(
                        "p (r d) -> p d r", r=1 + N_CTX
                    ),
                    axis=mybir.AxisListType.X,
                    op=mybir.AluOpType.add,
                )
                # scores for all 6 noise slots at once:
                # prod6[p, k, d] = x[p, d] * g[p, (9+k)*VEC + d]; reduce over d
                prod6 = gpool.tile([P, N_NOISE * VEC], mybir.dt.float32, tag="prod6")
                nc.vector.tensor_tensor(
                    out=prod6[:].rearrange("p (k d) -> p k d", k=N_NOISE),
                    in0=x[:, None, :].to_broadcast([P, N_NOISE, VEC]),
                    in1=g[:, (1 + N_CTX) * VEC : RPI * VEC].rearrange(
                        "p (k d) -> p k d", k=N_NOISE
                    ),
                    op=mybir.AluOpType.mult,
                )
                nc.vector.tensor_reduce(
                    out=score_t[:, j * N_NOISE : (j + 1) * N_NOISE],
                    in_=prod6[:].rearrange("p (k d) -> p k d", k=N_NOISE),
                    axis=mybir.AxisListType.X,
                    op=mybir.AluOpType.add,
                )
            nc.sync.dma_start(out=out[:], in_=score_t[:])
    nc.compile()
    return nc


def get_nc():
    global _nc_cache
    if _nc_cache is None:
        _nc_cache = build_nc()
    return _nc_cache


def make_host_inputs(context_ids, doc_ids, target_noise_ids, D, W, O):
    """Returns (tbl [200000,256] f32, per-core idx tiles [8][128, 60] i32)."""
    tbl = np.concatenate(
        [
            np.asarray(D, dtype=np.float32),
            np.asarray(W, dtype=np.float32),
            np.ascontiguousarray(np.asarray(O, dtype=np.float32).T),
        ],
        axis=0,
    )
    doc = np.asarray(doc_ids, dtype=np.int64).reshape(B, 1)
    ctx = np.asarray(context_ids, dtype=np.int64) + N_DOCS
    noi = np.asarray(target_noise_ids, dtype=np.int64) + (N_DOCS + N_WORDS)
    rows = np.concatenate([doc, ctx, noi], axis=1).astype(np.int32)  # [B, 15]
    idx_cores = []
    for c in range(N_CORES):
        r = rows[c * BPC : (c + 1) * BPC]  # [512, 15]
        idx_cores.append(
            np.ascontiguousarray(
                r.reshape(TILES, P, RPI).transpose(1, 0, 2).reshape(P, TILES * RPI)
            )
        )
    return tbl, idx_cores


def unshard_output(outs):
    """outs: list of 8 arrays [128, 24] -> scores [4096, 6] f32."""
    parts = []
    for o in outs:
        parts.append(
            np.ascontiguousarray(
                np.asarray(o, dtype=np.float32)
                .reshape(P, TILES, N_NOISE)
                .transpose(1, 0, 2)
                .reshape(BPC, N_NOISE)
            )
        )
    return np.concatenate(parts, axis=0)


def _install_profile_hook():
    """The agent image lacks ``antenv.axon_hooks``; inject the 3-line shim so
    run_bass_kernel_spmd(trace=True) can find the NTFF hook (the actual
    profiling impl lives in trn_agent_boot.trn_boot)."""
    import types

    if "antenv.axon_hooks" in sys.modules:
        return
    import antenv
    from trn_agent_boot.trn_boot import _ntff_profile_via_ctypes

    mod = types.ModuleType("antenv.axon_hooks")
    _state = {"hook": _ntff_profile_via_ctypes("/opt/axon/libaxon_pjrt.so")}
    mod.set_axon_ntff_profile_hook = lambda h: _state.__setitem__("hook", h)
    mod.get_axon_ntff_profile_hook = lambda: _state["hook"]
    sys.modules["antenv.axon_hooks"] = mod
    antenv.axon_hooks = mod


def kernel(context_ids, doc_ids, target_noise_ids, D, W, O, _trace=False):
    if _trace:
        _install_profile_hook()
    nc = get_nc()
    tbl, idx_cores = make_host_inputs(
        context_ids, doc_ids, target_noise_ids, D, W, O
    )
    in_maps = [{"tbl": tbl, "idx": idx_cores[c]} for c in range(N_CORES)]
    res = run_bass_kernel_spmd(
        nc, in_maps, core_ids=list(range(N_CORES)), trace=_trace
    )
    scores = unshard_output([res.results[c]["out"] for c in range(N_CORES)])
    if _trace:
        kernel.last_exec_time_ns = res.exec_time_ns
        kernel.last_results = res
    return scores


# revision 8
# speedup vs baseline: 1.1641x; 1.1641x over previous
"""Doc2vec-style embedding lookup + negative-sampling scores on 8 trn2 cores.

reference:
    x[b, :] = D[doc_ids[b]] + sum_c W[context_ids[b, c]]      # (B, 256)
    scores[b, k] = dot(x[b], O[:, target_noise_ids[b, k]])    # (B, 6)

Strategy: data-parallel over batch (512 items/core), tables replicated.
Host concatenates [D; W; O.T] into one row table so every lookup is a row
gather from a single DRAM tensor; each batch item needs 15 rows
(1 doc + 8 ctx + 6 noise).  Per core: 4 batch-tiles of 128 items; each tile
is ONE indirect DMA gathering 128x15 rows into SBUF, then a DVE strided
tensor_reduce sums the 9 embedding rows into x, and 6 fused
tensor_tensor_reduce ops produce the dot-product scores.
"""

import sys

sys.path.insert(0, "/opt/trn_rl_repo")

import numpy as np

from concourse import bacc, bass, mybir, tile
from concourse.bass_utils import run_bass_kernel_spmd

VEC = 256
N_DOCS = 100000
N_WORDS = 50000
B = 4096
N_CTX = 8
N_NOISE = 6
N_CORES = 8
BPC = B // N_CORES  # 512 batch items per core
P = 128
TILES = BPC // P  # 4 batch tiles per core
RPI = 1 + N_CTX + N_NOISE  # 15 gathered rows per batch item
T_ROWS = N_DOCS + 2 * N_WORDS  # 200000

_nc_cache = None


def build_nc():
    nc = bacc.Bacc(None, target_bir_lowering=False, debug=True)
    tbl = nc.declare_dram_parameter(
        "tbl", [T_ROWS, VEC], mybir.dt.float32, isOutput=False
    )
    idx = nc.declare_dram_parameter(
        "idx", [P, TILES * RPI], mybir.dt.int32, isOutput=False
    )
    out = nc.declare_dram_parameter(
        "out", [P, TILES * N_NOISE], mybir.dt.float32, isOutput=True
    )

    with tile.TileContext(nc) as tc:
        with (
            tc.tile_pool(name="gpool", bufs=TILES) as gpool,
            tc.tile_pool(name="vpool", bufs=2) as vpool,
            tc.tile_pool(name="cpool", bufs=1) as cpool,
        ):
            idx_t = cpool.tile([P, TILES * RPI], mybir.dt.int32)
            nc.sync.dma_start(out=idx_t[:], in_=idx[:])
            score_t = cpool.tile([P, TILES * N_NOISE], mybir.dt.float32)
            for j in range(TILES):
                g = gpool.tile([P, RPI * VEC], mybir.dt.float32, tag="g")
                # HW indirect DMA uses ONE index per partition (the rest of
                # the offset AP's free dim is ignored and the descriptor just
                # reads contiguous bytes), so emit one gather per row-slot.
                for r in range(RPI):
                    col = j * RPI + r
                    nc.gpsimd.indirect_dma_start(
                        out=g[:, r * VEC : (r + 1) * VEC],
                        out_offset=None,
                        in_=tbl[:],
                        in_offset=bass.IndirectOffsetOnAxis(
                            ap=idx_t[:, col : col + 1], axis=0
                        ),
                    )
                x = vpool.tile([P, VEC], mybir.dt.float32, tag="x")
                # x[p, d] = sum_r g[p, r*VEC + d] over the 9 embedding rows
                nc.vector.tensor_reduce(
                    out=x[:],
                    in_=g[:, : (1 + N_CTX) * VEC].rearrange(
                        "p (r d) -> p d r", r=1 + N_CTX
                    ),
                    axis=mybir.AxisListType.X,
                    op=mybir.AluOpType.add,
                )
                # scores for all 6 noise slots at once:
                # prod6[p, k, d] = x[p, d] * g[p, (9+k)*VEC + d]; reduce over d
                prod6 = vpool.tile([P, N_NOISE * VEC], mybir.dt.float32, tag="prod6")
                nc.vector.tensor_tensor(
                    out=prod6[:].rearrange("p (k d) -> p k d", k=N_NOISE),
                    in0=x[:, None, :].to_broadcast([P, N_NOISE, VEC]),
                    in1=g[:, (1 + N_CTX) * VEC : RPI * VEC].rearrange(
                        "p (k d) -> p k d", k=N_NOISE
                    ),
                    op=mybir.AluOpType.mult,
                )
                nc.vector.tensor_reduce(
                    out=score_t[:, j * N_NOISE : (j + 1) * N_NOISE],
                    in_=prod6[:].rearrange("p (k d) -> p k d", k=N_NOISE),
                    axis=mybir.AxisListType.X,
                    op=mybir.AluOpType.add,
                )
            nc.sync.dma_start(out=out[:], in_=score_t[:])
    nc.compile()
    return nc


def get_nc():
    global _nc_cache
    if _nc_cache is None:
        _nc_cache = build_nc()
    return _nc_cache


def make_host_inputs(context_ids, doc_ids, target_noise_ids, D, W, O):
    """Returns (tbl [200000,256] f32, per-core idx tiles [8][128, 60] i32)."""
    tbl = np.concatenate(
        [
            np.asarray(D, dtype=np.float32),
            np.asarray(W, dtype=np.float32),
            np.ascontiguousarray(np.asarray(O, dtype=np.float32).T),
        ],
        axis=0,
    )
    doc = np.asarray(doc_ids, dtype=np.int64).reshape(B, 1)
    ctx = np.asarray(context_ids, dtype=np.int64) + N_DOCS
    noi = np.asarray(target_noise_ids, dtype=np.int64) + (N_DOCS + N_WORDS)
    rows = np.concatenate([doc, ctx, noi], axis=1).astype(np.int32)  # [B, 15]
    idx_cores = []
    for c in range(N_CORES):
        r = rows[c * BPC : (c + 1) * BPC]  # [512, 15]
        idx_cores.append(
            np.ascontiguousarray(
                r.reshape(TILES, P, RPI).transpose(1, 0, 2).reshape(P, TILES * RPI)
            )
        )
    return tbl, idx_cores


def unshard_output(outs):
    """outs: list of 8 arrays [128, 24] -> scores [4096, 6] f32."""
    parts = []
    for o in outs:
        parts.append(
            np.ascontiguousarray(
                np.asarray(o, dtype=np.float32)
                .reshape(P, TILES, N_NOISE)
                .transpose(1, 0, 2)
                .reshape(BPC, N_NOISE)
            )
        )
    return np.concatenate(parts, axis=0)


def _install_profile_hook():
    """The agent image lacks ``antenv.axon_hooks``; inject the 3-line shim so
    run_bass_kernel_spmd(trace=True) can find the NTFF hook (the actual
    profiling impl lives in trn_agent_boot.trn_boot)."""
    import types

    if "antenv.axon_hooks" in sys.modules:
        return
    import antenv
    from trn_agent_boot.trn_boot import _ntff_profile_via_ctypes

    mod = types.ModuleType("antenv.axon_hooks")
    _state = {"hook": _ntff_profile_via_ctypes("/opt/axon/libaxon_pjrt.so")}
    mod.set_axon_ntff_profile_hook = lambda h: _state.__setitem__("hook", h)
    mod.get_axon_ntff_profile_hook = lambda: _state["hook"]
    sys.modules["antenv.axon_hooks"] = mod
    antenv.axon_hooks = mod


def kernel(context_ids, doc_ids, target_noise_ids, D, W, O, _trace=False):
    if _trace:
        _install_profile_hook()
    nc = get_nc()
    tbl, idx_cores = make_host_inputs(
        context_ids, doc_ids, target_noise_ids, D, W, O
    )
    in_maps = [{"tbl": tbl, "idx": idx_cores[c]} for c in range(N_CORES)]
    res = run_bass_kernel_spmd(
        nc, in_maps, core_ids=list(range(N_CORES)), trace=_trace
    )
    scores = unshard_output([res.results[c]["out"] for c in range(N_CORES)])
    if _trace:
        kernel.last_exec_time_ns = res.exec_time_ns
        kernel.last_results = res
    return scores


# revision 12
# speedup vs baseline: 1.1740x; 1.0085x over previous
"""Doc2vec-style embedding lookup + negative-sampling scores on 8 trn2 cores.

reference:
    x[b, :] = D[doc_ids[b]] + sum_c W[context_ids[b, c]]      # (B, 256)
    scores[b, k] = dot(x[b], O[:, target_noise_ids[b, k]])    # (B, 6)

Strategy: data-parallel over batch (512 items/core), tables replicated.
Host concatenates [D; W; O.T] into one row table so every lookup is a row
gather from a single DRAM tensor; each batch item needs 15 rows
(1 doc + 8 ctx + 6 noise).  Per core: 4 batch-tiles of 128 items; each tile
is ONE indirect DMA gathering 128x15 rows into SBUF, then a DVE strided
tensor_reduce sums the 9 embedding rows into x, and 6 fused
tensor_tensor_reduce ops produce the dot-product scores.
"""

import sys

sys.path.insert(0, "/opt/trn_rl_repo")

from contextlib import ExitStack

import numpy as np

from concourse import bacc, bass, mybir, tile
from concourse.bass_utils import run_bass_kernel_spmd

VEC = 256
N_DOCS = 100000
N_WORDS = 50000
B = 4096
N_CTX = 8
N_NOISE = 6
N_CORES = 8
BPC = B // N_CORES  # 512 batch items per core
P = 128
TILES = BPC // P  # 4 batch tiles per core
RPI = 1 + N_CTX + N_NOISE  # 15 gathered rows per batch item
T_ROWS = N_DOCS + 2 * N_WORDS  # 200000

_nc_cache = None


def build_nc_raw():
    """Raw-Bass (no TileContext) pipeline: avoids Tile's ~7us preamble EVSEM
    butterfly, per-gather sem bookkeeping (~310ns/gather), and the end
    barrier.  Sync: per-batch-tile semaphores with exact counts (16 incs per
    DMA x 9 or 6 DMAs), so a sem reaching its target proves every SDMA engine
    finished that tile's descriptors."""
    nc = bass.Bass(target_bir_lowering=False, debug=False)
    tbl = nc.declare_dram_parameter(
        "tbl", [T_ROWS, VEC], mybir.dt.float32, isOutput=False
    )
    idx = nc.declare_dram_parameter(
        "idx", [P, TILES * RPI], mybir.dt.int32, isOutput=False
    )
    out = nc.declare_dram_parameter(
        "out", [P, TILES * N_NOISE], mybir.dt.float32, isOutput=True
    )

    with ExitStack() as ctx:
        block = ctx.enter_context(nc.Block())
        sem_idx = ctx.enter_context(nc.semaphore("sem_idx"))
        sem_x = [ctx.enter_context(nc.semaphore(f"sem_x{j}")) for j in range(TILES)]
        sem_n = [ctx.enter_context(nc.semaphore(f"sem_n{j}")) for j in range(TILES)]
        sem_vec = ctx.enter_context(nc.semaphore("sem_vec"))
        sem_out = ctx.enter_context(nc.semaphore("sem_out"))
        idx_t = ctx.enter_context(
            nc.sbuf_tensor("idx_t", [P, TILES * RPI], mybir.dt.int32)
        )
        gbuf = ctx.enter_context(
            nc.sbuf_tensor("gbuf", [P, TILES * RPI * VEC], mybir.dt.float32)
        )
        x4 = ctx.enter_context(nc.sbuf_tensor("x4", [P, TILES * VEC], mybir.dt.float32))
        prod6 = ctx.enter_context(
            nc.sbuf_tensor("prod6", [P, N_NOISE * VEC], mybir.dt.float32)
        )
        score = ctx.enter_context(
            nc.sbuf_tensor("score", [P, TILES * N_NOISE], mybir.dt.float32)
        )

        @block.sync
        def _(s: bass.BassEngine):
            s.dma_start(out=idx_t[:, :], in_=idx[:, :]).then_inc(sem_idx, 16)
            s.wait_ge(sem_vec, 1)
            s.dma_start(out=out[:, :], in_=score[:, :]).then_inc(sem_out, 16)
            s.wait_ge(sem_out, 16)

        @block.gpsimd
        def _(g: bass.BassGpSimd):
            g.wait_ge(sem_idx, 16)
            for j in range(TILES):
                for r in range(RPI):
                    col = j * RPI + r
                    g.indirect_dma_start(
                        out=gbuf[:, col * VEC : (col + 1) * VEC],
                        out_offset=None,
                        in_=tbl[:],
                        in_offset=bass.IndirectOffsetOnAxis(
                            ap=idx_t[:, col : col + 1], axis=0
                        ),
                    ).then_inc(sem_x[j] if r <= N_CTX else sem_n[j], 16)

        @block.vector
        def _(v: bass.BassVectorEngine):
            for j in range(TILES):
                v.wait_ge(sem_x[j], (1 + N_CTX) * 16)
                v.tensor_reduce(
                    out=x4[:, j * VEC : (j + 1) * VEC],
                    in_=gbuf[
                        :, j * RPI * VEC : (j * RPI + 1 + N_CTX) * VEC
                    ].rearrange("p (r d) -> p d r", r=1 + N_CTX),
                    axis=mybir.AxisListType.X,
                    op=mybir.AluOpType.add,
                )
                v.drain()  # retire x4 write before tt reads it
                v.wait_ge(sem_n[j], N_NOISE * 16)
                v.tensor_tensor(
                    out=prod6[:, :].rearrange("p (k d) -> p k d", k=N_NOISE),
                    in0=x4[:, j * VEC : (j + 1) * VEC][:, None, :].to_broadcast(
                        [P, N_NOISE, VEC]
                    ),
                    in1=gbuf[
                        :, (j * RPI + 1 + N_CTX) * VEC : (j + 1) * RPI * VEC
                    ].rearrange("p (k d) -> p k d", k=N_NOISE),
                    op=mybir.AluOpType.mult,
                )
                v.drain()  # retire prod6 write before reduce reads it
                v.tensor_reduce(
                    out=score[:, j * N_NOISE : (j + 1) * N_NOISE],
                    in_=prod6[:, :].rearrange("p (k d) -> p k d", k=N_NOISE),
                    axis=mybir.AxisListType.X,
                    op=mybir.AluOpType.add,
                )
                v.drain()  # retire score/prod6 before next-j reuse / final store
            v.drain().then_inc(sem_vec, 1)

    return nc


def build_nc():
    nc = bacc.Bacc(None, target_bir_lowering=False, debug=True)
    tbl = nc.declare_dram_parameter(
        "tbl", [T_ROWS, VEC], mybir.dt.float32, isOutput=False
    )
    idx = nc.declare_dram_parameter(
        "idx", [P, TILES * RPI], mybir.dt.int32, isOutput=False
    )
    out = nc.declare_dram_parameter(
        "out", [P, TILES * N_NOISE], mybir.dt.float32, isOutput=True
    )

    with tile.TileContext(nc) as tc:
        with (
            tc.tile_pool(name="gpool", bufs=TILES) as gpool,
            tc.tile_pool(name="vpool", bufs=2) as vpool,
            tc.tile_pool(name="cpool", bufs=1) as cpool,
        ):
            idx_t = cpool.tile([P, TILES * RPI], mybir.dt.int32)
            nc.sync.dma_start(out=idx_t[:], in_=idx[:])
            score_t = cpool.tile([P, TILES * N_NOISE], mybir.dt.float32)
            for j in range(TILES):
                g = gpool.tile([P, RPI * VEC], mybir.dt.float32, tag="g")
                # HW indirect DMA uses ONE index per partition (the rest of
                # the offset AP's free dim is ignored and the descriptor just
                # reads contiguous bytes), so emit one gather per row-slot.
                for r in range(RPI):
                    col = j * RPI + r
                    nc.gpsimd.indirect_dma_start(
                        out=g[:, r * VEC : (r + 1) * VEC],
                        out_offset=None,
                        in_=tbl[:],
                        in_offset=bass.IndirectOffsetOnAxis(
                            ap=idx_t[:, col : col + 1], axis=0
                        ),
                    )
                x = vpool.tile([P, VEC], mybir.dt.float32, tag="x")
                # x[p, d] = sum_r g[p, r*VEC + d] over the 9 embedding rows
                nc.vector.tensor_reduce(
                    out=x[:],
                    in_=g[:, : (1 + N_CTX) * VEC].rearrange(
                        "p (r d) -> p d r", r=1 + N_CTX
                    ),
                    axis=mybir.AxisListType.X,
                    op=mybir.AluOpType.add,
                )
                # scores for all 6 noise slots at once:
                # prod6[p, k, d] = x[p, d] * g[p, (9+k)*VEC + d]; reduce over d
                prod6 = vpool.tile([P, N_NOISE * VEC], mybir.dt.float32, tag="prod6")
                nc.vector.tensor_tensor(
                    out=prod6[:].rearrange("p (k d) -> p k d", k=N_NOISE),
                    in0=x[:, None, :].to_broadcast([P, N_NOISE, VEC]),
                    in1=g[:, (1 + N_CTX) * VEC : RPI * VEC].rearrange(
                        "p (k d) -> p k d", k=N_NOISE
                    ),
                    op=mybir.AluOpType.mult,
                )
                nc.vector.tensor_reduce(
                    out=score_t[:, j * N_NOISE : (j + 1) * N_NOISE],
                    in_=prod6[:].rearrange("p (k d) -> p k d", k=N_NOISE),
                    axis=mybir.AxisListType.X,
                    op=mybir.AluOpType.add,
                )
            nc.sync.dma_start(out=out[:], in_=score_t[:])
    nc.compile()
    return nc


def get_nc():
    global _nc_cache
    if _nc_cache is None:
        _nc_cache = build_nc_raw()
    return _nc_cache


def make_host_inputs(context_ids, doc_ids, target_noise_ids, D, W, O):
    """Returns (tbl [200000,256] f32, per-core idx tiles [8][128, 60] i32)."""
    tbl = np.concatenate(
        [
            np.asarray(D, dtype=np.float32),
            np.asarray(W, dtype=np.float32),
            np.ascontiguousarray(np.asarray(O, dtype=np.float32).T),
        ],
        axis=0,
    )
    doc = np.asarray(doc_ids, dtype=np.int64).reshape(B, 1)
    ctx = np.asarray(context_ids, dtype=np.int64) + N_DOCS
    noi = np.asarray(target_noise_ids, dtype=np.int64) + (N_DOCS + N_WORDS)
    rows = np.concatenate([doc, ctx, noi], axis=1).astype(np.int32)  # [B, 15]
    idx_cores = []
    for c in range(N_CORES):
        r = rows[c * BPC : (c + 1) * BPC]  # [512, 15]
        idx_cores.append(
            np.ascontiguousarray(
                r.reshape(TILES, P, RPI).transpose(1, 0, 2).reshape(P, TILES * RPI)
            )
        )
    return tbl, idx_cores


def unshard_output(outs):
    """outs: list of 8 arrays [128, 24] -> scores [4096, 6] f32."""
    parts = []
    for o in outs:
        parts.append(
            np.ascontiguousarray(
                np.asarray(o, dtype=np.float32)
                .reshape(P, TILES, N_NOISE)
                .transpose(1, 0, 2)
                .reshape(BPC, N_NOISE)
            )
        )
    return np.concatenate(parts, axis=0)


def _install_profile_hook():
    """The agent image lacks ``antenv.axon_hooks``; inject the 3-line shim so
    run_bass_kernel_spmd(trace=True) can find the NTFF hook (the actual
    profiling impl lives in trn_agent_boot.trn_boot)."""
    import types

    if "antenv.axon_hooks" in sys.modules:
        return
    import antenv
    from trn_agent_boot.trn_boot import _ntff_profile_via_ctypes

    mod = types.ModuleType("antenv.axon_hooks")
    _state = {"hook": _ntff_profile_via_ctypes("/opt/axon/libaxon_pjrt.so")}
    mod.set_axon_ntff_profile_hook = lambda h: _state.__setitem__("hook", h)
    mod.get_axon_ntff_profile_hook = lambda: _state["hook"]
    sys.modules["antenv.axon_hooks"] = mod
    antenv.axon_hooks = mod


def kernel(context_ids, doc_ids, target_noise_ids, D, W, O, _trace=False):
    if _trace:
        _install_profile_hook()
    nc = get_nc()
    tbl, idx_cores = make_host_inputs(
        context_ids, doc_ids, target_noise_ids, D, W, O
    )
    in_maps = [{"tbl": tbl, "idx": idx_cores[c]} for c in range(N_CORES)]
    res = run_bass_kernel_spmd(
        nc, in_maps, core_ids=list(range(N_CORES)), trace=_trace
    )
    scores = unshard_output([res.results[c]["out"] for c in range(N_CORES)])
    if _trace:
        kernel.last_exec_time_ns = res.exec_time_ns
        kernel.last_results = res
    return scores


# revision 15
# speedup vs baseline: 1.1760x; 1.0017x over previous
"""Doc2vec-style embedding lookup + negative-sampling scores on 8 trn2 cores.

reference:
    x[b, :] = D[doc_ids[b]] + sum_c W[context_ids[b, c]]      # (B, 256)
    scores[b, k] = dot(x[b], O[:, target_noise_ids[b, k]])    # (B, 6)

Strategy: data-parallel over batch (512 items/core), tables replicated.
Host concatenates [D; W; O.T] into one row table so every lookup is a row
gather from a single DRAM tensor; each batch item needs 15 rows
(1 doc + 8 ctx + 6 noise).  Per core: 4 batch-tiles of 128 items; each tile
is ONE indirect DMA gathering 128x15 rows into SBUF, then a DVE strided
tensor_reduce sums the 9 embedding rows into x, and 6 fused
tensor_tensor_reduce ops produce the dot-product scores.
"""

import sys

sys.path.insert(0, "/opt/trn_rl_repo")

from contextlib import ExitStack

import numpy as np

from concourse import bacc, bass, mybir, tile
from concourse.bass_utils import run_bass_kernel_spmd

VEC = 256
N_DOCS = 100000
N_WORDS = 50000
B = 4096
N_CTX = 8
N_NOISE = 6
N_CORES = 8
BPC = B // N_CORES  # 512 batch items per core
P = 128
TILES = BPC // P  # 4 batch tiles per core
RPI = 1 + N_CTX + N_NOISE  # 15 gathered rows per batch item
T_ROWS = N_DOCS + 2 * N_WORDS  # 200000

_nc_cache = None


def build_nc_raw():
    """Raw-Bass (no TileContext) pipeline: avoids Tile's ~7us preamble EVSEM
    butterfly, per-gather sem bookkeeping (~310ns/gather), and the end
    barrier.  Sync: per-batch-tile semaphores with exact counts (16 incs per
    DMA x 9 or 6 DMAs), so a sem reaching its target proves every SDMA engine
    finished that tile's descriptors."""
    nc = bass.Bass(target_bir_lowering=False, debug=False, num_swdge_queues=2)
    tbl = nc.declare_dram_parameter(
        "tbl", [T_ROWS, VEC], mybir.dt.float32, isOutput=False
    )
    idx = nc.declare_dram_parameter(
        "idx", [P, TILES * RPI], mybir.dt.int32, isOutput=False
    )
    out = nc.declare_dram_parameter(
        "out", [P, TILES * N_NOISE], mybir.dt.float32, isOutput=True
    )

    with ExitStack() as ctx:
        block = ctx.enter_context(nc.Block(no_gpsimd_drain=True))
        sem_idx = ctx.enter_context(nc.semaphore("sem_idx"))
        sem_x = [ctx.enter_context(nc.semaphore(f"sem_x{j}")) for j in range(TILES)]
        sem_n = [ctx.enter_context(nc.semaphore(f"sem_n{j}")) for j in range(TILES)]
        sem_vec = ctx.enter_context(nc.semaphore("sem_vec"))
        sem_out = ctx.enter_context(nc.semaphore("sem_out"))
        idx_t = ctx.enter_context(
            nc.sbuf_tensor("idx_t", [P, TILES * RPI], mybir.dt.int32)
        )
        gbuf = ctx.enter_context(
            nc.sbuf_tensor("gbuf", [P, TILES * RPI * VEC], mybir.dt.float32)
        )
        x4 = ctx.enter_context(nc.sbuf_tensor("x4", [P, TILES * VEC], mybir.dt.float32))
        prod6 = ctx.enter_context(
            nc.sbuf_tensor("prod6", [P, N_NOISE * VEC], mybir.dt.float32)
        )
        score = ctx.enter_context(
            nc.sbuf_tensor("score", [P, TILES * N_NOISE], mybir.dt.float32)
        )

        @block.sync
        def _(s: bass.BassEngine):
            s.dma_start(out=idx_t[:, :], in_=idx[:, :]).then_inc(sem_idx, 16)
            s.wait_ge(sem_vec, 1)
            s.dma_start(out=out[:, :], in_=score[:, :]).then_inc(sem_out, 16)
            s.wait_ge(sem_out, 16)

        @block.gpsimd
        def _(g: bass.BassGpSimd):
            g.wait_ge(sem_idx, 16)
            for j in range(TILES):
                for r in range(RPI):
                    col = j * RPI + r
                    ins = g.indirect_dma_start(
                        out=gbuf[:, col * VEC : (col + 1) * VEC],
                        out_offset=None,
                        in_=tbl[:],
                        in_offset=bass.IndirectOffsetOnAxis(
                            ap=idx_t[:, col : col + 1], axis=0
                        ),
                    )
                    ins.then_inc(sem_x[j] if r <= N_CTX else sem_n[j], 16)
                    if col % 2 == 1:
                        ins.queue = "qPoolDynamic1"

        @block.vector
        def _(v: bass.BassVectorEngine):
            for j in range(TILES):
                v.wait_ge(sem_x[j], (1 + N_CTX) * 16)
                v.tensor_reduce(
                    out=x4[:, j * VEC : (j + 1) * VEC],
                    in_=gbuf[
                        :, j * RPI * VEC : (j * RPI + 1 + N_CTX) * VEC
                    ].rearrange("p (r d) -> p d r", r=1 + N_CTX),
                    axis=mybir.AxisListType.X,
                    op=mybir.AluOpType.add,
                )
                v.drain()  # retire x4 write before tt reads it
                v.wait_ge(sem_n[j], N_NOISE * 16)
                v.tensor_tensor(
                    out=prod6[:, :].rearrange("p (k d) -> p k d", k=N_NOISE),
                    in0=x4[:, j * VEC : (j + 1) * VEC][:, None, :].to_broadcast(
                        [P, N_NOISE, VEC]
                    ),
                    in1=gbuf[
                        :, (j * RPI + 1 + N_CTX) * VEC : (j + 1) * RPI * VEC
                    ].rearrange("p (k d) -> p k d", k=N_NOISE),
                    op=mybir.AluOpType.mult,
                )
                v.drain()  # retire prod6 write before reduce reads it
                v.tensor_reduce(
                    out=score[:, j * N_NOISE : (j + 1) * N_NOISE],
                    in_=prod6[:, :].rearrange("p (k d) -> p k d", k=N_NOISE),
                    axis=mybir.AxisListType.X,
                    op=mybir.AluOpType.add,
                )
                v.drain()  # retire score/prod6 before next-j reuse / final store
            v.drain().then_inc(sem_vec, 1)

    return nc


def build_nc():
    nc = bacc.Bacc(None, target_bir_lowering=False, debug=True)
    tbl = nc.declare_dram_parameter(
        "tbl", [T_ROWS, VEC], mybir.dt.float32, isOutput=False
    )
    idx = nc.declare_dram_parameter(
        "idx", [P, TILES * RPI], mybir.dt.int32, isOutput=False
    )
    out = nc.declare_dram_parameter(
        "out", [P, TILES * N_NOISE], mybir.dt.float32, isOutput=True
    )

    with tile.TileContext(nc) as tc:
        with (
            tc.tile_pool(name="gpool", bufs=TILES) as gpool,
            tc.tile_pool(name="vpool", bufs=2) as vpool,
            tc.tile_pool(name="cpool", bufs=1) as cpool,
        ):
            idx_t = cpool.tile([P, TILES * RPI], mybir.dt.int32)
            nc.sync.dma_start(out=idx_t[:], in_=idx[:])
            score_t = cpool.tile([P, TILES * N_NOISE], mybir.dt.float32)
            for j in range(TILES):
                g = gpool.tile([P, RPI * VEC], mybir.dt.float32, tag="g")
                # HW indirect DMA uses ONE index per partition (the rest of
                # the offset AP's free dim is ignored and the descriptor just
                # reads contiguous bytes), so emit one gather per row-slot.
                for r in range(RPI):
                    col = j * RPI + r
                    nc.gpsimd.indirect_dma_start(
                        out=g[:, r * VEC : (r + 1) * VEC],
                        out_offset=None,
                        in_=tbl[:],
                        in_offset=bass.IndirectOffsetOnAxis(
                            ap=idx_t[:, col : col + 1], axis=0
                        ),
                    )
                x = vpool.tile([P, VEC], mybir.dt.float32, tag="x")
                # x[p, d] = sum_r g[p, r*VEC + d] over the 9 embedding rows
                nc.vector.tensor_reduce(
                    out=x[:],
                    in_=g[:, : (1 + N_CTX) * VEC].rearrange(
                        "p (r d) -> p d r", r=1 + N_CTX
                    ),
                    axis=mybir.AxisListType.X,
                    op=mybir.AluOpType.add,
                )
                # scores for all 6 noise slots at once:
                # prod6[p, k, d] = x[p, d] * g[p, (9+k)*VEC + d]; reduce over d
                prod6 = vpool.tile([P, N_NOISE * VEC], mybir.dt.float32, tag="prod6")
                nc.vector.tensor_tensor(
                    out=prod6[:].rearrange("p (k d) -> p k d", k=N_NOISE),
                    in0=x[:, None, :].to_broadcast([P, N_NOISE, VEC]),
                    in1=g[:, (1 + N_CTX) * VEC : RPI * VEC].rearrange(
                        "p (k d) -> p k d", k=N_NOISE
                    ),
                    op=mybir.AluOpType.mult,
                )
                nc.vector.tensor_reduce(
                    out=score_t[:, j * N_NOISE : (j + 1) * N_NOISE],
                    in_=prod6[:].rearrange("p (k d) -> p k d", k=N_NOISE),
                    axis=mybir.AxisListType.X,
                    op=mybir.AluOpType.add,
                )
            nc.sync.dma_start(out=out[:], in_=score_t[:])
    nc.compile()
    return nc


def get_nc():
    global _nc_cache
    if _nc_cache is None:
        _nc_cache = build_nc_raw()
    return _nc_cache


def make_host_inputs(context_ids, doc_ids, target_noise_ids, D, W, O):
    """Returns (tbl [200000,256] f32, per-core idx tiles [8][128, 60] i32)."""
    tbl = np.concatenate(
        [
            np.asarray(D, dtype=np.float32),
            np.asarray(W, dtype=np.float32),
            np.ascontiguousarray(np.asarray(O, dtype=np.float32).T),
        ],
        axis=0,
    )
    doc = np.asarray(doc_ids, dtype=np.int64).reshape(B, 1)
    ctx = np.asarray(context_ids, dtype=np.int64) + N_DOCS
    noi = np.asarray(target_noise_ids, dtype=np.int64) + (N_DOCS + N_WORDS)
    rows = np.concatenate([doc, ctx, noi], axis=1).astype(np.int32)  # [B, 15]
    idx_cores = []
    for c in range(N_CORES):
        r = rows[c * BPC : (c + 1) * BPC]  # [512, 15]
        idx_cores.append(
            np.ascontiguousarray(
                r.reshape(TILES, P, RPI).transpose(1, 0, 2).reshape(P, TILES * RPI)
            )
        )
    return tbl, idx_cores


def unshard_output(outs):
    """outs: list of 8 arrays [128, 24] -> scores [4096, 6] f32."""
    parts = []
    for o in outs:
        parts.append(
            np.ascontiguousarray(
                np.asarray(o, dtype=np.float32)
                .reshape(P, TILES, N_NOISE)
                .transpose(1, 0, 2)
                .reshape(BPC, N_NOISE)
            )
        )
    return np.concatenate(parts, axis=0)


def _install_profile_hook():
    """The agent image lacks ``antenv.axon_hooks``; inject the 3-line shim so
    run_bass_kernel_spmd(trace=True) can find the NTFF hook (the actual
    profiling impl lives in trn_agent_boot.trn_boot)."""
    import types

    if "antenv.axon_hooks" in sys.modules:
        return
    import antenv
    from trn_agent_boot.trn_boot import _ntff_profile_via_ctypes

    mod = types.ModuleType("antenv.axon_hooks")
    _state = {"hook": _ntff_profile_via_ctypes("/opt/axon/libaxon_pjrt.so")}
    mod.set_axon_ntff_profile_hook = lambda h: _state.__setitem__("hook", h)
    mod.get_axon_ntff_profile_hook = lambda: _state["hook"]
    sys.modules["antenv.axon_hooks"] = mod
    antenv.axon_hooks = mod


def kernel(context_ids, doc_ids, target_noise_ids, D, W, O, _trace=False):
    if _trace:
        _install_profile_hook()
    nc = get_nc()
    tbl, idx_cores = make_host_inputs(
        context_ids, doc_ids, target_noise_ids, D, W, O
    )
    in_maps = [{"tbl": tbl, "idx": idx_cores[c]} for c in range(N_CORES)]
    res = run_bass_kernel_spmd(
        nc, in_maps, core_ids=list(range(N_CORES)), trace=_trace
    )
    scores = unshard_output([res.results[c]["out"] for c in range(N_CORES)])
    if _trace:
        kernel.last_exec_time_ns = res.exec_time_ns
        kernel.last_results = res
    return scores


# revision 19
# speedup vs baseline: 1.2064x; 1.0258x over previous
"""Doc2vec-style embedding lookup + negative-sampling scores on 8 trn2 cores.

reference:
    x[b, :] = D[doc_ids[b]] + sum_c W[context_ids[b, c]]      # (B, 256)
    scores[b, k] = dot(x[b], O[:, target_noise_ids[b, k]])    # (B, 6)

Strategy: data-parallel over batch (512 items/core), tables replicated.
Host concatenates [D; W; O.T] into one row table so every lookup is a row
gather from a single DRAM tensor; each batch item needs 15 rows
(1 doc + 8 ctx + 6 noise).  Per core: 4 batch-tiles of 128 items; each tile
is ONE indirect DMA gathering 128x15 rows into SBUF, then a DVE strided
tensor_reduce sums the 9 embedding rows into x, and 6 fused
tensor_tensor_reduce ops produce the dot-product scores.
"""

import sys

sys.path.insert(0, "/opt/trn_rl_repo")

from contextlib import ExitStack

import numpy as np

from concourse import bacc, bass, mybir, tile
from concourse.bass_utils import run_bass_kernel_spmd

VEC = 256
N_DOCS = 100000
N_WORDS = 50000
B = 4096
N_CTX = 8
N_NOISE = 6
N_CORES = 8
BPC = B // N_CORES  # 512 batch items per core
P = 128
TILES = BPC // P  # 4 batch tiles per core
RPI = 1 + N_CTX + N_NOISE  # 15 gathered rows per batch item
T_ROWS = N_DOCS + 2 * N_WORDS  # 200000

_nc_cache = None


def build_nc_raw():
    """Raw-Bass (no TileContext) pipeline: avoids Tile's ~7us preamble EVSEM
    butterfly, per-gather sem bookkeeping (~310ns/gather), and the end
    barrier.  Sync: per-batch-tile semaphores with exact counts (16 incs per
    DMA x 9 or 6 DMAs), so a sem reaching its target proves every SDMA engine
    finished that tile's descriptors."""
    nc = bass.Bass(target_bir_lowering=False, debug=False, num_swdge_queues=2)
    tbl = nc.declare_dram_parameter(
        "tbl", [T_ROWS, VEC], mybir.dt.float32, isOutput=False
    )
    idx = nc.declare_dram_parameter(
        "idx", [P, TILES * RPI], mybir.dt.int32, isOutput=False
    )
    out = nc.declare_dram_parameter(
        "out", [P, TILES * N_NOISE], mybir.dt.float32, isOutput=True
    )

    with ExitStack() as ctx:
        block = ctx.enter_context(nc.Block(no_gpsimd_drain=True))
        sem_idx = ctx.enter_context(nc.semaphore("sem_idx"))
        sem_x = [ctx.enter_context(nc.semaphore(f"sem_x{j}")) for j in range(TILES)]
        sem_n = [
            [
                ctx.enter_context(nc.semaphore(f"sem_n{j}_{h}"))
                for h in range(2)
            ]
            for j in range(TILES)
        ]
        sem_vec = ctx.enter_context(nc.semaphore("sem_vec"))
        sem_out = ctx.enter_context(nc.semaphore("sem_out"))
        idx_t = ctx.enter_context(
            nc.sbuf_tensor("idx_t", [P, TILES * RPI], mybir.dt.int32)
        )
        gbuf = ctx.enter_context(
            nc.sbuf_tensor("gbuf", [P, TILES * RPI * VEC], mybir.dt.float32)
        )
        x4 = ctx.enter_context(nc.sbuf_tensor("x4", [P, TILES * VEC], mybir.dt.float32))
        prod6 = ctx.enter_context(
            nc.sbuf_tensor("prod6", [P, N_NOISE * VEC], mybir.dt.float32)
        )
        score = ctx.enter_context(
            nc.sbuf_tensor("score", [P, TILES * N_NOISE], mybir.dt.float32)
        )

        @block.sync
        def _(s: bass.BassEngine):
            s.dma_start(out=idx_t[:, :], in_=idx[:, :]).then_inc(sem_idx, 16)
            s.wait_ge(sem_vec, 1)
            s.dma_start(out=out[:, :], in_=score[:, :]).then_inc(sem_out, 16)
            s.wait_ge(sem_out, 16)

        @block.gpsimd
        def _(g: bass.BassGpSimd):
            g.wait_ge(sem_idx, 16)
            for j in range(TILES):
                for r in range(RPI):
                    col = j * RPI + r
                    ins = g.indirect_dma_start(
                        out=gbuf[:, col * VEC : (col + 1) * VEC],
                        out_offset=None,
                        in_=tbl[:],
                        in_offset=bass.IndirectOffsetOnAxis(
                            ap=idx_t[:, col : col + 1], axis=0
                        ),
                    )
                    if r <= N_CTX:
                        tgt = sem_x[j]
                    else:
                        tgt = sem_n[j][(r - 1 - N_CTX) // (N_NOISE // 2)]
                    ins.then_inc(tgt, 16)
                    if col % 2 == 1:
                        ins.queue = "qPoolDynamic1"

        @block.vector
        def _(v: bass.BassVectorEngine):
            for j in range(TILES):
                v.wait_ge(sem_x[j], (1 + N_CTX) * 16)
                v.tensor_reduce(
                    out=x4[:, j * VEC : (j + 1) * VEC],
                    in_=gbuf[
                        :, j * RPI * VEC : (j * RPI + 1 + N_CTX) * VEC
                    ].rearrange("p (r d) -> p d r", r=1 + N_CTX),
                    axis=mybir.AxisListType.X,
                    op=mybir.AluOpType.add,
                )
                v.drain()  # retire x4 write before tt reads it
                # noise scores in two 3-slot chunks so the last chunk's DVE
                # work after the final gather is half-sized
                half = N_NOISE // 2
                for h in range(2):
                    k0 = h * half
                    v.wait_ge(sem_n[j][h], half * 16)
                    pslice = prod6[:, k0 * VEC : (k0 + half) * VEC]
                    v.tensor_tensor(
                        out=pslice.rearrange("p (k d) -> p k d", k=half),
                        in0=x4[:, j * VEC : (j + 1) * VEC][:, None, :].to_broadcast(
                            [P, half, VEC]
                        ),
                        in1=gbuf[
                            :,
                            (j * RPI + 1 + N_CTX + k0) * VEC : (
                                j * RPI + 1 + N_CTX + k0 + half
                            )
                            * VEC,
                        ].rearrange("p (k d) -> p k d", k=half),
                        op=mybir.AluOpType.mult,
                    )
                    v.drain()  # retire prod6 chunk before reduce reads it
                    v.tensor_reduce(
                        out=score[:, j * N_NOISE + k0 : j * N_NOISE + k0 + half],
                        in_=pslice.rearrange("p (k d) -> p k d", k=half),
                        axis=mybir.AxisListType.X,
                        op=mybir.AluOpType.add,
                    )
                v.drain()  # retire score before next-j reuse / final store
            v.drain().then_inc(sem_vec, 1)

    return nc


def build_nc():
    nc = bacc.Bacc(None, target_bir_lowering=False, debug=True)
    tbl = nc.declare_dram_parameter(
        "tbl", [T_ROWS, VEC], mybir.dt.float32, isOutput=False
    )
    idx = nc.declare_dram_parameter(
        "idx", [P, TILES * RPI], mybir.dt.int32, isOutput=False
    )
    out = nc.declare_dram_parameter(
        "out", [P, TILES * N_NOISE], mybir.dt.float32, isOutput=True
    )

    with tile.TileContext(nc) as tc:
        with (
            tc.tile_pool(name="gpool", bufs=TILES) as gpool,
            tc.tile_pool(name="vpool", bufs=2) as vpool,
            tc.tile_pool(name="cpool", bufs=1) as cpool,
        ):
            idx_t = cpool.tile([P, TILES * RPI], mybir.dt.int32)
            nc.sync.dma_start(out=idx_t[:], in_=idx[:])
            score_t = cpool.tile([P, TILES * N_NOISE], mybir.dt.float32)
            for j in range(TILES):
                g = gpool.tile([P, RPI * VEC], mybir.dt.float32, tag="g")
                # HW indirect DMA uses ONE index per partition (the rest of
                # the offset AP's free dim is ignored and the descriptor just
                # reads contiguous bytes), so emit one gather per row-slot.
                for r in range(RPI):
                    col = j * RPI + r
                    nc.gpsimd.indirect_dma_start(
                        out=g[:, r * VEC : (r + 1) * VEC],
                        out_offset=None,
                        in_=tbl[:],
                        in_offset=bass.IndirectOffsetOnAxis(
                            ap=idx_t[:, col : col + 1], axis=0
                        ),
                    )
                x = vpool.tile([P, VEC], mybir.dt.float32, tag="x")
                # x[p, d] = sum_r g[p, r*VEC + d] over the 9 embedding rows
                nc.vector.tensor_reduce(
                    out=x[:],
                    in_=g[:, : (1 + N_CTX) * VEC].rearrange(
                        "p (r d) -> p d r", r=1 + N_CTX
                    ),
                    axis=mybir.AxisListType.X,
                    op=mybir.AluOpType.add,
                )
                # scores for all 6 noise slots at once:
                # prod6[p, k, d] = x[p, d] * g[p, (9+k)*VEC + d]; reduce over d
                prod6 = vpool.tile([P, N_NOISE * VEC], mybir.dt.float32, tag="prod6")
                nc.vector.tensor_tensor(
                    out=prod6[:].rearrange("p (k d) -> p k d", k=N_NOISE),
                    in0=x[:, None, :].to_broadcast([P, N_NOISE, VEC]),
                    in1=g[:, (1 + N_CTX) * VEC : RPI * VEC].rearrange(
                        "p (k d) -> p k d", k=N_NOISE
                    ),
                    op=mybir.AluOpType.mult,
                )
                nc.vector.tensor_reduce(
                    out=score_t[:, j * N_NOISE : (j + 1) * N_NOISE],
                    in_=prod6[:].rearrange("p (k d) -> p k d", k=N_NOISE),
                    axis=mybir.AxisListType.X,
                    op=mybir.AluOpType.add,
                )
            nc.sync.dma_start(out=out[:], in_=score_t[:])
    nc.compile()
    return nc


def get_nc():
    global _nc_cache
    if _nc_cache is None:
        _nc_cache = build_nc_raw()
    return _nc_cache


def make_host_inputs(context_ids, doc_ids, target_noise_ids, D, W, O):
    """Returns (tbl [200000,256] f32, per-core idx tiles [8][128, 60] i32)."""
    tbl = np.concatenate(
        [
            np.asarray(D, dtype=np.float32),
            np.asarray(W, dtype=np.float32),
            np.ascontiguousarray(np.asarray(O, dtype=np.float32).T),
        ],
        axis=0,
    )
    doc = np.asarray(doc_ids, dtype=np.int64).reshape(B, 1)
    ctx = np.asarray(context_ids, dtype=np.int64) + N_DOCS
    noi = np.asarray(target_noise_ids, dtype=np.int64) + (N_DOCS + N_WORDS)
    rows = np.concatenate([doc, ctx, noi], axis=1).astype(np.int32)  # [B, 15]
    idx_cores = []
    for c in range(N_CORES):
        r = rows[c * BPC : (c + 1) * BPC]  # [512, 15]
        idx_cores.append(
            np.ascontiguousarray(
                r.reshape(TILES, P, RPI).transpose(1, 0, 2).reshape(P, TILES * RPI)
            )
        )
    return tbl, idx_cores


def unshard_output(outs):
    """outs: list of 8 arrays [128, 24] -> scores [4096, 6] f32."""
    parts = []
    for o in outs:
        parts.append(
            np.ascontiguousarray(
                np.asarray(o, dtype=np.float32)
                .reshape(P, TILES, N_NOISE)
                .transpose(1, 0, 2)
                .reshape(BPC, N_NOISE)
            )
        )
    return np.concatenate(parts, axis=0)


def _install_profile_hook():
    """The agent image lacks ``antenv.axon_hooks``; inject the 3-line shim so
    run_bass_kernel_spmd(trace=True) can find the NTFF hook (the actual
    profiling impl lives in trn_agent_boot.trn_boot)."""
    import types

    if "antenv.axon_hooks" in sys.modules:
        return
    import antenv
    from trn_agent_boot.trn_boot import _ntff_profile_via_ctypes

    mod = types.ModuleType("antenv.axon_hooks")
    _state = {"hook": _ntff_profile_via_ctypes("/opt/axon/libaxon_pjrt.so")}
    mod.set_axon_ntff_profile_hook = lambda h: _state.__setitem__("hook", h)
    mod.get_axon_ntff_profile_hook = lambda: _state["hook"]
    sys.modules["antenv.axon_hooks"] = mod
    antenv.axon_hooks = mod


def kernel(context_ids, doc_ids, target_noise_ids, D, W, O, _trace=False):
    if _trace:
        _install_profile_hook()
    nc = get_nc()
    tbl, idx_cores = make_host_inputs(
        context_ids, doc_ids, target_noise_ids, D, W, O
    )
    in_maps = [{"tbl": tbl, "idx": idx_cores[c]} for c in range(N_CORES)]
    res = run_bass_kernel_spmd(
        nc, in_maps, core_ids=list(range(N_CORES)), trace=_trace
    )
    scores = unshard_output([res.results[c]["out"] for c in range(N_CORES)])
    if _trace:
        kernel.last_exec_time_ns = res.exec_time_ns
        kernel.last_results = res
    return scores


# revision 27
# speedup vs baseline: 1.2123x; 1.0049x over previous
"""Doc2vec-style embedding lookup + negative-sampling scores on 8 trn2 cores.

reference:
    x[b, :] = D[doc_ids[b]] + sum_c W[context_ids[b, c]]      # (B, 256)
    scores[b, k] = dot(x[b], O[:, target_noise_ids[b, k]])    # (B, 6)

Strategy: data-parallel over batch (512 items/core), tables replicated.
Host concatenates [D; W; O.T] into one row table so every lookup is a row
gather from a single DRAM tensor; each batch item needs 15 rows
(1 doc + 8 ctx + 6 noise).  Per core: 4 batch-tiles of 128 items; each tile
is ONE indirect DMA gathering 128x15 rows into SBUF, then a DVE strided
tensor_reduce sums the 9 embedding rows into x, and 6 fused
tensor_tensor_reduce ops produce the dot-product scores.
"""

import sys

sys.path.insert(0, "/opt/trn_rl_repo")

from contextlib import ExitStack

import numpy as np

from concourse import bacc, bass, mybir, tile
from concourse.bass_utils import run_bass_kernel_spmd

VEC = 256
N_DOCS = 100000
N_WORDS = 50000
B = 4096
N_CTX = 8
N_NOISE = 6
N_CORES = 8
BPC = B // N_CORES  # 512 batch items per core
P = 128
TILES = BPC // P  # 4 batch tiles per core
RPI = 1 + N_CTX + N_NOISE  # 15 gathered rows per batch item
T_ROWS = N_DOCS + 2 * N_WORDS  # 200000

_nc_cache = None


def build_nc_raw():
    """Raw-Bass (no TileContext) pipeline: avoids Tile's ~7us preamble EVSEM
    butterfly, per-gather sem bookkeeping (~310ns/gather), and the end
    barrier.  Sync: per-batch-tile semaphores with exact counts (16 incs per
    DMA x 9 or 6 DMAs), so a sem reaching its target proves every SDMA engine
    finished that tile's descriptors."""
    nc = bass.Bass(target_bir_lowering=False, debug=False, num_swdge_queues=2)
    tbl = nc.declare_dram_parameter(
        "tbl", [T_ROWS, VEC], mybir.dt.float32, isOutput=False
    )
    idx = nc.declare_dram_parameter(
        "idx", [P, TILES * RPI], mybir.dt.int32, isOutput=False
    )
    out = nc.declare_dram_parameter(
        "out", [P, TILES * N_NOISE], mybir.dt.float32, isOutput=True
    )

    with ExitStack() as ctx:
        block = ctx.enter_context(nc.Block(no_gpsimd_drain=True))
        sem_idx = ctx.enter_context(nc.semaphore("sem_idx"))
        sem_x = [ctx.enter_context(nc.semaphore(f"sem_x{j}")) for j in range(TILES)]
        sem_n = [
            [
                ctx.enter_context(nc.semaphore(f"sem_n{j}_{h}"))
                for h in range(2)
            ]
            for j in range(TILES)
        ]
        sem_vec = ctx.enter_context(nc.semaphore("sem_vec"))
        sem_out = ctx.enter_context(nc.semaphore("sem_out"))
        idx_t = ctx.enter_context(
            nc.sbuf_tensor("idx_t", [P, TILES * RPI], mybir.dt.int32)
        )
        gbuf = ctx.enter_context(
            nc.sbuf_tensor("gbuf", [P, TILES * RPI * VEC], mybir.dt.float32)
        )
        x4 = ctx.enter_context(nc.sbuf_tensor("x4", [P, TILES * VEC], mybir.dt.float32))
        prod6 = ctx.enter_context(
            nc.sbuf_tensor("prod6", [P, N_NOISE * VEC], mybir.dt.float32)
        )
        score = ctx.enter_context(
            nc.sbuf_tensor("score", [P, TILES * N_NOISE], mybir.dt.float32)
        )

        @block.sync
        def _(s: bass.BassEngine):
            s.dma_start(out=idx_t[:, :], in_=idx[:, :]).then_inc(sem_idx, 16)
            s.wait_ge(sem_vec, 1)
            s.dma_start(out=out[:, :], in_=score[:, :]).then_inc(sem_out, 16)
            s.wait_ge(sem_out, 16)

        @block.gpsimd
        def _(g: bass.BassGpSimd):
            g.wait_ge(sem_idx, 16)
            for j in range(TILES):
                for r in range(RPI):
                    col = j * RPI + r
                    ins = g.indirect_dma_start(
                        out=gbuf[:, col * VEC : (col + 1) * VEC],
                        out_offset=None,
                        in_=tbl[:],
                        in_offset=bass.IndirectOffsetOnAxis(
                            ap=idx_t[:, col : col + 1], axis=0
                        ),
                    )
                    if r <= N_CTX:
                        tgt = sem_x[j]
                    else:
                        tgt = sem_n[j][(r - 1 - N_CTX) // (N_NOISE // 2)]
                    ins.then_inc(tgt, 16)
                    if col % 2 == 1:
                        ins.queue = "qPoolDynamic1"

        @block.vector
        def _(v: bass.BassVectorEngine):
            for j in range(TILES):
                v.wait_ge(sem_x[j], (1 + N_CTX) * 16)
                v.tensor_reduce(
                    out=x4[:, j * VEC : (j + 1) * VEC],
                    in_=gbuf[
                        :, j * RPI * VEC : (j * RPI + 1 + N_CTX) * VEC
                    ].rearrange("p (r d) -> p d r", r=1 + N_CTX),
                    axis=mybir.AxisListType.X,
                    op=mybir.AluOpType.add,
                )
                v.drain()  # retire x4 write before tt reads it
                # noise scores in two 3-slot chunks so the last chunk's DVE
                # work after the final gather is half-sized
                half = N_NOISE // 2
                for h in range(2):
                    k0 = h * half
                    v.wait_ge(sem_n[j][h], half * 16)
                    pslice = prod6[:, k0 * VEC : (k0 + half) * VEC]
                    v.tensor_tensor(
                        out=pslice.rearrange("p (k d) -> p k d", k=half),
                        in0=x4[:, j * VEC : (j + 1) * VEC][:, None, :].to_broadcast(
                            [P, half, VEC]
                        ),
                        in1=gbuf[
                            :,
                            (j * RPI + 1 + N_CTX + k0) * VEC : (
                                j * RPI + 1 + N_CTX + k0 + half
                            )
                            * VEC,
                        ].rearrange("p (k d) -> p k d", k=half),
                        op=mybir.AluOpType.mult,
                    )
                    v.drain()  # retire prod6 chunk before reduce reads it
                    v.tensor_reduce(
                        out=score[:, j * N_NOISE + k0 : j * N_NOISE + k0 + half],
                        in_=pslice.rearrange("p (k d) -> p k d", k=half),
                        axis=mybir.AxisListType.X,
                        op=mybir.AluOpType.add,
                    )
                v.drain()  # retire score before next-j reuse / final store
            v.drain().then_inc(sem_vec, 1)

    return nc


def build_nc():
    nc = bacc.Bacc(None, target_bir_lowering=False, debug=True)
    tbl = nc.declare_dram_parameter(
        "tbl", [T_ROWS, VEC], mybir.dt.float32, isOutput=False
    )
    idx = nc.declare_dram_parameter(
        "idx", [P, TILES * RPI], mybir.dt.int32, isOutput=False
    )
    out = nc.declare_dram_parameter(
        "out", [P, TILES * N_NOISE], mybir.dt.float32, isOutput=True
    )

    with tile.TileContext(nc) as tc:
        with (
            tc.tile_pool(name="gpool", bufs=TILES) as gpool,
            tc.tile_pool(name="vpool", bufs=2) as vpool,
            tc.tile_pool(name="cpool", bufs=1) as cpool,
        ):
            idx_t = cpool.tile([P, TILES * RPI], mybir.dt.int32)
            nc.sync.dma_start(out=idx_t[:], in_=idx[:])
            score_t = cpool.tile([P, TILES * N_NOISE], mybir.dt.float32)
            for j in range(TILES):
                g = gpool.tile([P, RPI * VEC], mybir.dt.float32, tag="g")
                # HW indirect DMA uses ONE index per partition (the rest of
                # the offset AP's free dim is ignored and the descriptor just
                # reads contiguous bytes), so emit one gather per row-slot.
                for r in range(RPI):
                    col = j * RPI + r
                    nc.gpsimd.indirect_dma_start(
                        out=g[:, r * VEC : (r + 1) * VEC],
                        out_offset=None,
                        in_=tbl[:],
                        in_offset=bass.IndirectOffsetOnAxis(
                            ap=idx_t[:, col : col + 1], axis=0
                        ),
                    )
                x = vpool.tile([P, VEC], mybir.dt.float32, tag="x")
                # x[p, d] = sum_r g[p, r*VEC + d] over the 9 embedding rows
                nc.vector.tensor_reduce(
                    out=x[:],
                    in_=g[:, : (1 + N_CTX) * VEC].rearrange(
                        "p (r d) -> p d r", r=1 + N_CTX
                    ),
                    axis=mybir.AxisListType.X,
                    op=mybir.AluOpType.add,
                )
                # scores for all 6 noise slots at once:
                # prod6[p, k, d] = x[p, d] * g[p, (9+k)*VEC + d]; reduce over d
                prod6 = vpool.tile([P, N_NOISE * VEC], mybir.dt.float32, tag="prod6")
                nc.vector.tensor_tensor(
                    out=prod6[:].rearrange("p (k d) -> p k d", k=N_NOISE),
                    in0=x[:, None, :].to_broadcast([P, N_NOISE, VEC]),
                    in1=g[:, (1 + N_CTX) * VEC : RPI * VEC].rearrange(
                        "p (k d) -> p k d", k=N_NOISE
                    ),
                    op=mybir.AluOpType.mult,
                )
                nc.vector.tensor_reduce(
                    out=score_t[:, j * N_NOISE : (j + 1) * N_NOISE],
                    in_=prod6[:].rearrange("p (k d) -> p k d", k=N_NOISE),
                    axis=mybir.AxisListType.X,
                    op=mybir.AluOpType.add,
                )
            nc.sync.dma_start(out=out[:], in_=score_t[:])
    nc.compile()
    return nc


def get_nc():
    global _nc_cache
    if _nc_cache is None:
        _nc_cache = build_nc_raw()
    return _nc_cache


def make_host_inputs(context_ids, doc_ids, target_noise_ids, D, W, O):
    """Returns (tbl [200000,256] f32, per-core idx tiles [8][128, 60] i32)."""
    tbl = np.concatenate(
        [
            np.asarray(D, dtype=np.float32),
            np.asarray(W, dtype=np.float32),
            np.ascontiguousarray(np.asarray(O, dtype=np.float32).T),
        ],
        axis=0,
    )
    doc = np.asarray(doc_ids, dtype=np.int64).reshape(B, 1)
    ctx = np.asarray(context_ids, dtype=np.int64) + N_DOCS
    noi = np.asarray(target_noise_ids, dtype=np.int64) + (N_DOCS + N_WORDS)
    rows = np.concatenate([doc, ctx, noi], axis=1).astype(np.int32)  # [B, 15]
    idx_cores = []
    for c in range(N_CORES):
        r = rows[c * BPC : (c + 1) * BPC]  # [512, 15]
        idx_cores.append(
            np.ascontiguousarray(
                r.reshape(TILES, P, RPI).transpose(1, 0, 2).reshape(P, TILES * RPI)
            )
        )
    return tbl, idx_cores


def unshard_output(outs):
    """outs: list of 8 arrays [128, 24] -> scores [4096, 6] f32."""
    parts = []
    for o in outs:
        parts.append(
            np.ascontiguousarray(
                np.asarray(o, dtype=np.float32)
                .reshape(P, TILES, N_NOISE)
                .transpose(1, 0, 2)
                .reshape(BPC, N_NOISE)
            )
        )
    return np.concatenate(parts, axis=0)


def _install_profile_hook():
    """The agent image lacks ``antenv.axon_hooks``; inject the 3-line shim so
    run_bass_kernel_spmd(trace=True) can find the NTFF hook (the actual
    profiling impl lives in trn_agent_boot.trn_boot)."""
    import types

    if "antenv.axon_hooks" in sys.modules:
        return
    import antenv
    from trn_agent_boot.trn_boot import _ntff_profile_via_ctypes

    mod = types.ModuleType("antenv.axon_hooks")
    _state = {"hook": _ntff_profile_via_ctypes("/opt/axon/libaxon_pjrt.so")}
    mod.set_axon_ntff_profile_hook = lambda h: _state.__setitem__("hook", h)
    mod.get_axon_ntff_profile_hook = lambda: _state["hook"]
    sys.modules["antenv.axon_hooks"] = mod
    antenv.axon_hooks = mod


def kernel(context_ids, doc_ids, target_noise_ids, D, W, O, _trace=False):
    if _trace:
        _install_profile_hook()
    nc = get_nc()
    tbl, idx_cores = make_host_inputs(
        context_ids, doc_ids, target_noise_ids, D, W, O
    )
    in_maps = [{"tbl": tbl, "idx": idx_cores[c]} for c in range(N_CORES)]
    res = run_bass_kernel_spmd(
        nc, in_maps, core_ids=list(range(N_CORES)), trace=_trace
    )
    scores = unshard_output([res.results[c]["out"] for c in range(N_CORES)])
    if _trace:
        kernel.last_exec_time_ns = res.exec_time_ns
        kernel.last_results = res
    return scores
